# revision 3
# baseline (speedup 1.0000x reference)
"""GATv2 (2-layer, 8-head) Trainium2 kernel, 8-core node-sharded.

Pipeline per layer:
  T-NEFF (per-core, sharded): node transforms xl = x@Wl+bl, xr = x@Wr+br
    via fp32r matmuls; emits bf16 gather tables (xl) and bf16 xr shards.
  host: assembles the full xl gather table from the 8 shards (data movement
    only), then
  E-NEFF (per-core, sharded by dst): per-edge score + segment-softmax +
    aggregate, with edges laid out stratum-major: edge slot (q, d) holds the
    q-th in-edge of dst-slot d, so partition index == dst slot.  The
    xr broadcast is a plain broadcast AP, segment aggregation is a PSUM
    accumulation of identity matmuls, and segment max/sum are free-dim
    reduces.  xl[src] rows are fetched with gpsimd dma_gather (int16
    indices, so the node table is split at 32768 and each block gathers
    from both halves into disjoint strata).

Between the two layers the host only concatenates/transposes shards.
"""

import os
from contextlib import ExitStack

import ml_dtypes
import numpy as np

N, E0, DIN, H, DH, DOUT = 50000, 1600000, 128, 8, 16, 7
F1 = H * DH            # 128
F2P = 64               # layer-2 per-node feature block, 8 heads x 8 (7 real)
NCORES = 8
P = 128
NBLK = 392             # 392*128 = 50176 >= N, 392 % 8 == 0
NB = NBLK // NCORES    # 49 blocks per core
NOWN = NB * P          # 6272 nodes per core (incl. pad slots)
NPAD = NBLK * P        # 50176
SPLIT = 32768
TABB_ROWS = NPAD - SPLIT  # 17408
NEG = -1.0e9
EPS = 1e-16

_f32 = np.float32
_bf16 = ml_dtypes.bfloat16


# ---------------------------------------------------------------------------
# host-side graph preprocessing (pure index/layout manipulation)
# ---------------------------------------------------------------------------

def _prep_graph(edge_index):
    src = np.concatenate([edge_index[0], np.arange(N, dtype=np.int64)])
    dst = np.concatenate([edge_index[1], np.arange(N, dtype=np.int64)])
    src = src.astype(np.int64)
    dst = dst.astype(np.int64)

    low = src < SPLIT
    l_cnt = np.bincount(dst[low], minlength=N).astype(np.int64)
    h_cnt = np.bincount(dst[~low], minlength=N).astype(np.int64)

    # group nodes into blocks of 128 with near-equal (low-deg, high-deg)
    order = np.lexsort((h_cnt, l_cnt))
    nodes_sorted = np.concatenate([order, np.full(NPAD - N, -1, np.int64)])
    blocks = nodes_sorted.reshape(NBLK, P)          # [392, 128]

    l_blk = np.where(blocks >= 0, l_cnt[np.maximum(blocks, 0)], 0).max(axis=1)
    h_blk = np.where(blocks >= 0, h_cnt[np.maximum(blocks, 0)], 0).max(axis=1)
    # block-slot j on every core runs global blocks j*8+k; shared strata counts
    GA = l_blk.reshape(NB, NCORES).max(axis=1).astype(int)   # [49]
    GB = h_blk.reshape(NB, NCORES).max(axis=1).astype(int)
    # round up to even so dma_gather's num_idxs register values stay few
    # (bass caches one Pool register per distinct immediate)
    GA = np.where(GA > 0, (GA + 1) & ~1, 0)
    GB = np.where(GB > 0, (GB + 1) & ~1, 0)

    # per-node padded src lists, split by src half
    key = dst * 2 + (~low).astype(np.int64)
    oe = np.argsort(key, kind="stable")
    ss, sk = src[oe], key[oe]
    starts = np.searchsorted(sk, np.arange(2 * N))
    pos = np.arange(len(ss)) - starts[sk]
    Amax = max(int(l_cnt.max()), int(GA.max()))
    Bmax = max(int(h_cnt.max()), int(GB.max()))
    A_pad = np.zeros((N, Amax), np.int32)
    B_pad = np.zeros((N, Bmax), np.int32)
    am = (sk % 2) == 0
    A_pad[sk[am] // 2, pos[am]] = ss[am]
    B_pad[sk[~am] // 2, pos[~am]] = ss[~am] - SPLIT

    sumGA, sumGB = int(GA.sum()), int(GB.sum())
    sumG = sumGA + sumGB

    members = [None] * NCORES
    idxA = [None] * NCORES
    idxB = [None] * NCORES
    mneg = [None] * NCORES

    for k in range(NCORES):
        mem = blocks[np.arange(NB) * NCORES + k]       # [49, 128]
        members[k] = mem
        ia = np.zeros((P, 8 * sumGA), np.int16)
        ib = np.zeros((P, 8 * sumGB), np.int16)
        mg = np.full((P, sumG), NEG, _f32)
        oa = ob = om = 0
        for j in range(NB):
            ga, gb = GA[j], GB[j]
            m = mem[j]
            msafe = np.maximum(m, 0)
            larr = np.where(m >= 0, l_cnt[msafe], 0)
            harr = np.where(m >= 0, h_cnt[msafe], 0)
            if ga:
                plane = A_pad[msafe, :ga]              # [128, ga] (d, q)
                flat = plane.T.reshape(-1)             # slot-major (q, d)
                ia[:, 8 * oa:8 * (oa + ga)] = np.tile(
                    flat.reshape(-1, 16).T, (8, 1)).astype(np.int16)
                mg[:, om:om + ga] = np.where(
                    np.arange(ga)[None, :] < larr[:, None], 0.0, NEG)
            if gb:
                plane = B_pad[msafe, :gb]
                flat = plane.T.reshape(-1)
                ib[:, 8 * ob:8 * (ob + gb)] = np.tile(
                    flat.reshape(-1, 16).T, (8, 1)).astype(np.int16)
                mg[:, om + ga:om + ga + gb] = np.where(
                    np.arange(gb)[None, :] < harr[:, None], 0.0, NEG)
            oa += ga
            ob += gb
            om += ga + gb
        idxA[k], idxB[k], mneg[k] = ia, ib, mg

    return dict(members=members, GA=GA, GB=GB, idxA=idxA, idxB=idxB,
                mneg=mneg, sumGA=sumGA, sumGB=sumGB, sumG=sumG)


# ---------------------------------------------------------------------------
# NEFF builders
# ---------------------------------------------------------------------------

def _mk_bass():
    import concourse.bacc as bacc
    return bacc.Bacc("TRN2", target_bir_lowering=False)


def _build_transform(fo, xl_cols, xl_w, xr_w, fast_mm):
    """xT [128, NOWN] @ Wcat [128, fo] -> xl rows (bf16) + xr rows (bf16).

    xl tensor is [NOWN, xl_cols]; only cols [0:xl_w] are written (rest
    stays zero).  xr tensor is [NOWN, xr_w]."""
    import concourse.mybir as mybir
    import concourse.tile as tile

    nc = _mk_bass()
    BF16, F32, F32R = mybir.dt.bfloat16, mybir.dt.float32, mybir.dt.float32r
    xT = nc.dram_tensor("xT", [P, NOWN], F32, kind="ExternalInput")
    W = nc.dram_tensor("Wcat", [P, fo], F32, kind="ExternalInput")
    B = nc.dram_tensor("Bcat", [P, fo], F32, kind="ExternalInput")
    xl = nc.dram_tensor("xl", [NOWN, xl_cols], BF16, kind="ExternalOutput")
    xr = nc.dram_tensor("xr", [NOWN, xr_w], BF16, kind="ExternalOutput")

    with tile.TileContext(nc) as tc, ExitStack() as ctx:
        const = ctx.enter_context(tc.tile_pool(name="const", bufs=1))
        work = ctx.enter_context(tc.tile_pool(name="work", bufs=3))
        psum = ctx.enter_context(tc.tile_pool(name="psum", bufs=2, space="PSUM"))

        w_f = const.tile([P, fo], F32)
        nc.sync.dma_start(w_f[:], W[:, :])
        w_s = const.tile([P, fo], BF16)
        nc.vector.tensor_copy(w_s[:], w_f[:])
        b_s = const.tile([P, fo], F32)
        nc.sync.dma_start(b_s[:], B[:, :])

        for j in range(NB):
            lhs_f = work.tile([P, P], F32, tag="lhsf")
            nc.sync.dma_start(lhs_f[:], xT[:, j * P:(j + 1) * P])
            lhs = work.tile([P, P], BF16, tag="lhs")
            nc.vector.tensor_copy(lhs[:], lhs_f[:])
            ps = psum.tile([P, fo], F32, tag="ps")
            nc.tensor.matmul(ps[:], lhs[:], w_s[:], start=True, stop=True)
            ol = work.tile([P, xl_w], BF16, tag="ol")
            nc.vector.tensor_tensor(ol[:], ps[:, 0:xl_w], b_s[:, 0:xl_w],
                                    mybir.AluOpType.add)
            orr = work.tile([P, xr_w], BF16, tag="orr")
            nc.vector.tensor_tensor(orr[:], ps[:, xl_w:fo], b_s[:, xl_w:fo],
                                    mybir.AluOpType.add)
            nc.sync.dma_start(xl[j * P:(j + 1) * P, 0:xl_w], ol[:])
            nc.sync.dma_start(xr[j * P:(j + 1) * P, :], orr[:])
    nc.compile()
    return nc


def _build_edge(layer, GA, GB, sumGA, sumGB, sumG):
    """Edge phase for one layer (see module docstring)."""
    import concourse.bass as bass
    import concourse.mybir as mybir
    import concourse.tile as tile
    from concourse import library_config

    FU = F1 if layer == 1 else F2P      # used feature cols (128 / 64)
    C = DH if layer == 1 else 8         # per-head cols in slab (16 / 8)
    FM = FU + H                         # matmul rhs cols (agg | denom)
    FOUT = F1 if layer == 1 else H * DOUT

    nc = _mk_bass()
    dt = mybir.dt
    op = mybir.AluOpType
    AF = mybir.ActivationFunctionType

    tabA = nc.dram_tensor("tabA", [SPLIT, P], dt.bfloat16, kind="ExternalInput")
    tabB = nc.dram_tensor("tabB", [TABB_ROWS, P], dt.bfloat16, kind="ExternalInput")
    xr_d = nc.dram_tensor("xr", [NOWN, FU], dt.bfloat16, kind="ExternalInput")
    idxA = nc.dram_tensor("idxA", [P, 8 * sumGA], dt.int16, kind="ExternalInput")
    idxB = nc.dram_tensor("idxB", [P, 8 * sumGB], dt.int16, kind="ExternalInput")
    mneg = nc.dram_tensor("mneg", [P, sumG], dt.float32, kind="ExternalInput")
    attT = nc.dram_tensor("attT", [P, FU], dt.bfloat16, kind="ExternalInput")
    biasT = nc.dram_tensor("biasT", [P, FU], dt.float32, kind="ExternalInput")
    idT = nc.dram_tensor("idT", [P, P], dt.bfloat16, kind="ExternalInput")
    out_d = nc.dram_tensor("out", [NOWN, FOUT], dt.float32, kind="ExternalOutput")

    with tile.TileContext(nc) as tc, ExitStack() as ctx:
        const = ctx.enter_context(tc.tile_pool(name="const", bufs=1))
        io = ctx.enter_context(tc.tile_pool(name="io", bufs=3))
        slabs = ctx.enter_context(tc.tile_pool(name="slabs", bufs=2))
        psum = ctx.enter_context(tc.tile_pool(name="psum", bufs=2, space="PSUM"))
        small = ctx.enter_context(tc.tile_pool(name="small", bufs=3))

        nc.gpsimd.load_library(library_config.mlp)

        regcache = {}

        def nreg(v):
            if v not in regcache:
                regcache[v] = nc.gpsimd.to_reg(v)
            return regcache[v]

        att_s = const.tile([P, FU], dt.bfloat16)
        nc.sync.dma_start(att_s[:], attT[:, :])
        bias_s = const.tile([P, FU], dt.float32)
        nc.sync.dma_start(bias_s[:], biasT[:, :])
        id_s = const.tile([P, P], dt.bfloat16)
        nc.sync.dma_start(id_s[:], idT[:, :])

        if layer == 2:
            persist = ctx.enter_context(tc.tile_pool(name="persist", bufs=1))
            mx_all = persist.tile([P, NB], dt.float32)
            s_all = persist.tile([P, NB], dt.float32)
            y_tiles = []

        oa = obi = om = orow = 0
        for j in range(NB):
            ga, gb = int(GA[j]), int(GB[j])
            g = ga + gb
            assert g > 0

            xr_b = io.tile([P, FU], dt.bfloat16, tag="xr")
            nc.sync.dma_start(xr_b[:], xr_d[j * P:(j + 1) * P, :])
            mg = io.tile([P, g], dt.float32, tag="mg")
            nc.sync.dma_start(mg[:], mneg[:, om:om + g])

            slab = slabs.tile([P, g, P], dt.bfloat16, tag="slab")
            if ga:
                ia = io.tile([P, 8 * ga], dt.int16, tag="ia")
                nc.sync.dma_start(ia[:], idxA[:, 8 * oa:8 * (oa + ga)])
                nc.gpsimd.dma_gather(slab[:, 0:ga, :], tabA[:, :], ia[:],
                                     P * ga, nreg(P * ga), P,
                                     single_packet=False)
            if gb:
                ib = io.tile([P, 8 * gb], dt.int16, tag="ib")
                nc.sync.dma_start(ib[:], idxB[:, 8 * obi:8 * (obi + gb)])
                nc.gpsimd.dma_gather(slab[:, ga:g, :], tabB[:, :], ib[:],
                                     P * gb, nreg(P * gb), P,
                                     single_packet=False)

            sl_u = slab[:, :, 0:FU]
            tt = slabs.tile([P, g, FU], dt.bfloat16, tag="tt")
            nc.vector.tensor_tensor(
                tt[:], sl_u, xr_b[:].unsqueeze(1).to_broadcast([P, g, FU]),
                op.add)
            uu = slabs.tile([P, g, FU], dt.bfloat16, tag="uu")
            nc.vector.scalar_tensor_tensor(uu[:], tt[:], 0.2, tt[:],
                                           op.mult, op.max)
            vv = slabs.tile([P, g, FU], dt.bfloat16, tag="tt")
            nc.vector.tensor_tensor(
                vv[:], uu[:], att_s[:].unsqueeze(1).to_broadcast([P, g, FU]),
                op.mult)

            sc = small.tile([P, g, H], dt.float32, tag="sc")
            nc.vector.tensor_reduce(
                sc[:], vv[:].rearrange("p g (h c) -> p g h c", c=C),
                mybir.AxisListType.X, op.add)
            sc2 = small.tile([P, g, H], dt.float32, tag="sc2")
            nc.vector.tensor_tensor(
                sc2[:], sc[:], mg[:].unsqueeze(2).to_broadcast([P, g, H]),
                op.add)
            mx = small.tile([P, H], dt.float32, tag="mx")
            nc.vector.tensor_reduce(
                mx[:], sc2[:].rearrange("p g h -> p h g"),
                mybir.AxisListType.X, op.max)
            sc3 = small.tile([P, g, H], dt.float32, tag="sc3")
            nc.vector.tensor_tensor(
                sc3[:], sc2[:], mx[:].unsqueeze(1).to_broadcast([P, g, H]),
                op.subtract)

            Ms = slabs.tile([P, g, FM], dt.bfloat16, tag="Ms")
            exv = Ms[:, :, FU:FM]
            nc.scalar.activation(exv, sc3[:], AF.Exp)
            nc.vector.tensor_tensor(
                Ms[:, :, 0:FU].rearrange("p g (h c) -> p g h c", c=C),
                sl_u.rearrange("p g (h c) -> p g h c", c=C),
                exv.unsqueeze(3).to_broadcast([P, g, H, C]),
                op.mult)

            ps = psum.tile([P, FM], dt.float32, tag="ps")
            for q in range(g):
                nc.tensor.matmul(ps[:], id_s[:], Ms[:, q, :],
                                 start=(q == 0), stop=(q == g - 1))

            dn = small.tile([P, H], dt.float32, tag="dn")
            nc.vector.tensor_scalar_add(dn[:], ps[:, FU:FM], EPS)
            rd = small.tile([P, H], dt.float32, tag="rd")
            nc.vector.reciprocal(rd[:], dn[:])
            ov = small.tile([P, FU], dt.float32, tag="ov")
            nc.vector.tensor_tensor(
                ov[:].rearrange("p (h c) -> p h c", c=C),
                ps[:, 0:FU].rearrange("p (h c) -> p h c", c=C),
                rd[:].unsqueeze(2).to_broadcast([P, H, C]),
                op.mult)
            ob = small.tile([P, FU], dt.float32, tag="ob")
            nc.vector.tensor_tensor(ob[:], ov[:], bias_s[:], op.add)

            if layer == 1:
                mm_t = small.tile([P, FU], dt.float32, tag="mmt")
                nc.vector.tensor_scalar_min(mm_t[:], ob[:], 0.0)
                em = small.tile([P, FU], dt.float32, tag="em")
                nc.scalar.activation(em[:], mm_t[:], AF.Exp)
                hf = small.tile([P, FU], dt.float32, tag="hf")
                nc.vector.scalar_tensor_tensor(
                    hf[:], ob[:], 0.0, em[:], op.max, op.add)
                hg = small.tile([P, FU], dt.float32, tag="hg")
                nc.vector.tensor_scalar_add(hg[:], hf[:], -1.0)
                nc.sync.dma_start(out_d[orow:orow + P, :], hg[:])
            else:
                yb = persist.tile([P, FU], dt.float32, tag=f"y{j}",
                                  name=f"y{j}")
                nc.vector.tensor_copy(yb[:], ob[:])
                yr = yb[:].rearrange("p (h c) -> p h c", c=8)[:, :, 0:DOUT]
                mx2 = mx_all[:, j:j + 1]
                nc.vector.tensor_reduce(mx2, yr, mybir.AxisListType.XY,
                                        op.max)
                mxn = small.tile([P, 1], dt.float32, tag="mxn")
                nc.vector.tensor_scalar_mul(mxn[:], mx2, -1.0)
                et = small.tile([P, FOUT], dt.float32, tag="et")
                nc.scalar.activation(
                    et[:].rearrange("p (h c) -> p h c", c=DOUT), yr,
                    AF.Exp, bias=mxn[:])
                nc.vector.tensor_reduce(s_all[:, j:j + 1], et[:],
                                        mybir.AxisListType.X, op.add)
                y_tiles.append(yb)

            oa += ga
            obi += gb
            om += g
            orow += P

        if layer == 2:
            # ln(S) via exponent/mantissa split (no Ln in any HW act table):
            # ln(S) = (e - 127)*ln2 + poly(m), m in [1, 2)
            C5, C4, C3, C2, C1, C0 = (0.030102625011658456,
                                      -0.2806325404494927,
                                      1.1048082361987304,
                                      -2.4208125632180866,
                                      3.4982279012091095,
                                      -1.9316715417207186)
            bits = s_all[:].bitcast(dt.int32)
            ei = persist.tile([P, NB], dt.int32)
            nc.vector.tensor_scalar(ei[:], bits, 23, None,
                                    op.arith_shift_right)
            ef = persist.tile([P, NB], dt.float32)
            nc.vector.tensor_copy(ef[:], ei[:])
            mi = persist.tile([P, NB], dt.int32)
            nc.vector.tensor_scalar(mi[:], bits, 0x007FFFFF, 0x3F800000,
                                    op.bitwise_and, op.bitwise_or)
            mf = mi[:].bitcast(dt.float32)
            pp = persist.tile([P, NB], dt.float32)
            nc.vector.tensor_scalar(pp[:], mf, C5, C4, op.mult, op.add)
            qq = persist.tile([P, NB], dt.float32)
            for ck in (C3, C2, C1, C0):
                nc.vector.tensor_tensor(qq[:], pp[:], mf, op.mult)
                nc.vector.tensor_scalar_add(pp[:], qq[:], ck)
            # ct = mx + (e-127)*ln2 + poly(m)
            lnm = pp
            ct_all = persist.tile([P, NB], dt.float32)
            nc.vector.scalar_tensor_tensor(
                ct_all[:], ef[:], 0.6931471805599453, lnm[:],
                op.mult, op.add)
            ct2 = persist.tile([P, NB], dt.float32)
            nc.vector.scalar_tensor_tensor(
                ct2[:], ct_all[:], -127.0 * 0.6931471805599453, mx_all[:],
                op.add, op.add)
            orow = 0
            for j in range(NB):
                yr = y_tiles[j][:].rearrange("p (h c) -> p h c",
                                             c=8)[:, :, 0:DOUT]
                of = small.tile([P, FOUT], dt.float32, tag="of")
                nc.vector.tensor_scalar_sub(
                    of[:].rearrange("p (h c) -> p h c", c=DOUT), yr,
                    ct2[:, j:j + 1])
                nc.sync.dma_start(out_d[orow:orow + P, :], of[:])
                orow += P
    nc.compile()
    return nc


# ---------------------------------------------------------------------------
# runner
# ---------------------------------------------------------------------------

_state = {}


def _run(nc, in_maps, trace=False):
    from concourse.bass_utils import run_bass_kernel_spmd
    return run_bass_kernel_spmd(nc, in_maps, core_ids=list(range(NCORES)),
                                trace=trace)


def _bcast_rows(v, rows=P):
    """[n] -> [rows, n] replicated, contiguous."""
    return np.ascontiguousarray(np.broadcast_to(np.asarray(v)[None, :],
                                                (rows, len(v))))


def kernel(x, edge_index, Wl1, bl1, Wr1, br1, att1, bias1,
           Wl2, bl2, Wr2, br2, att2, bias2, _trace=False, _times=None,
           _paths=None):
    x = np.asarray(x, _f32)
    edge_index = np.asarray(edge_index)

    g = _prep_graph(edge_index)
    members, GA, GB = g["members"], g["GA"], g["GB"]

    ckey = (tuple(GA), tuple(GB))
    if _state.get("ckey") != ckey:
        _state["ckey"] = ckey
        _state["nc_t1"] = _build_transform(2 * F1, F1, F1, F1, fast_mm=False)
        _state["nc_t2"] = _build_transform(2 * F2P, P, F2P, F2P, fast_mm=False)
        _state["nc_e1"] = _build_edge(1, GA, GB, g["sumGA"], g["sumGB"], g["sumG"])
        _state["nc_e2"] = _build_edge(2, GA, GB, g["sumGA"], g["sumGB"], g["sumG"])

    id128 = np.eye(P, dtype=_bf16)

    def gather_nodes(arr, mem):
        flat = mem.reshape(-1)
        out = arr[np.maximum(flat, 0)]
        out[flat < 0] = 0
        return out

    def trace_run(key, nc, in_maps):
        r = _run(nc, in_maps, trace=_trace)
        if _times is not None:
            _times[key] = r.exec_time_ns
        if _paths is not None and r.instructions_and_trace is not None:
            _paths[key] = r.instructions_and_trace[1]
        return r.results

    # ---- T1 ----
    W1 = np.concatenate([Wl1, Wr1], axis=1).astype(_f32)       # [128, 256]
    B1 = np.concatenate([bl1, br1]).astype(_f32)               # [256]
    B1t = _bcast_rows(B1)
    t1_maps = []
    for k in range(NCORES):
        xg = gather_nodes(x, members[k])                       # [6272, 128]
        t1_maps.append({"xT": np.ascontiguousarray(xg.T),
                        "Wcat": W1, "Bcat": B1t})
    r1 = trace_run("t1", _state["nc_t1"], t1_maps)

    # assemble layer-1 gather table
    tab1 = np.zeros((NPAD, P), _bf16)
    for k in range(NCORES):
        flat = members[k].reshape(-1)
        ok = flat >= 0
        tab1[flat[ok]] = r1[k]["xl"][ok]
    tab1A = np.ascontiguousarray(tab1[:SPLIT])
    tab1B = np.ascontiguousarray(tab1[SPLIT:])

    # ---- E1 ----
    att1_t = _bcast_rows(att1.reshape(-1)).astype(_bf16)       # [128, 128]
    bias1_t = _bcast_rows(bias1).astype(_f32)
    e1_maps = []
    for k in range(NCORES):
        e1_maps.append({"tabA": tab1A, "tabB": tab1B,
                        "xr": r1[k]["xr"],
                        "idxA": g["idxA"][k], "idxB": g["idxB"][k],
                        "mneg": g["mneg"][k],
                        "attT": att1_t, "biasT": bias1_t, "idT": id128})
    re1 = trace_run("e1", _state["nc_e1"], e1_maps)

    # ---- T2 ----
    Wl2p = np.zeros((P, F2P), _f32)
    Wl2p.reshape(P, H, 8)[:, :, :DOUT] = np.asarray(Wl2, _f32).reshape(P, H, DOUT)
    Wr2p = np.zeros((P, F2P), _f32)
    Wr2p.reshape(P, H, 8)[:, :, :DOUT] = np.asarray(Wr2, _f32).reshape(P, H, DOUT)
    W2 = np.ascontiguousarray(np.concatenate([Wl2p, Wr2p], axis=1))  # [128,128]
    bl2p = np.zeros(F2P, _f32)
    bl2p.reshape(H, 8)[:, :DOUT] = np.asarray(bl2, _f32).reshape(H, DOUT)
    br2p = np.zeros(F2P, _f32)
    br2p.reshape(H, 8)[:, :DOUT] = np.asarray(br2, _f32).reshape(H, DOUT)
    B2t = _bcast_rows(np.concatenate([bl2p, br2p]))
    t2_maps = []
    for k in range(NCORES):
        t2_maps.append({"xT": np.ascontiguousarray(re1[k]["out"].T),
                        "Wcat": W2, "Bcat": B2t})
    r2 = trace_run("t2", _state["nc_t2"], t2_maps)

    tab2 = np.zeros((NPAD, P), _bf16)
    for k in range(NCORES):
        flat = members[k].reshape(-1)
        ok = flat >= 0
        tab2[flat[ok]] = r2[k]["xl"][ok]
    tab2A = np.ascontiguousarray(tab2[:SPLIT])
    tab2B = np.ascontiguousarray(tab2[SPLIT:])

    # ---- E2 ----
    att2p = np.zeros((H, 8), _f32)
    att2p[:, :DOUT] = np.asarray(att2, _f32)
    att2_t = _bcast_rows(att2p.reshape(-1)).astype(_bf16)      # [128, 64]
    bias2p = np.zeros(F2P, _f32)
    bias2p.reshape(H, 8)[:, :DOUT] = np.asarray(bias2, _f32).reshape(H, DOUT)
    bias2_t = _bcast_rows(bias2p)
    e2_maps = []
    for k in range(NCORES):
        e2_maps.append({"tabA": tab2A, "tabB": tab2B,
                        "xr": r2[k]["xr"],
                        "idxA": g["idxA"][k], "idxB": g["idxB"][k],
                        "mneg": g["mneg"][k],
                        "attT": att2_t, "biasT": bias2_t, "idT": id128})
    re2 = trace_run("e2", _state["nc_e2"], e2_maps)

    out = np.zeros((N, H * DOUT), _f32)
    for k in range(NCORES):
        flat = members[k].reshape(-1)
        ok = flat >= 0
        out[flat[ok]] = re2[k]["out"][ok]
    return out



# revision 13
# speedup vs baseline: 1.4555x; 1.4555x over previous
"""GATv2 (2-layer, 8-head) Trainium2 kernel, 8-core node-sharded.

Pipeline per layer:
  T-NEFF (per-core, sharded): node transforms xl = x@Wl+bl, xr = x@Wr+br
    via fp32r matmuls; emits bf16 gather tables (xl) and bf16 xr shards.
  host: assembles the full xl gather table from the 8 shards (data movement
    only), then
  E-NEFF (per-core, sharded by dst): per-edge score + segment-softmax +
    aggregate, with edges laid out stratum-major: edge slot (q, d) holds the
    q-th in-edge of dst-slot d, so partition index == dst slot.  The
    xr broadcast is a plain broadcast AP, segment aggregation is a PSUM
    accumulation of identity matmuls, and segment max/sum are free-dim
    reduces.  xl[src] rows are fetched with gpsimd dma_gather (int16
    indices, so the node table is split at 32768 and each block gathers
    from both halves into disjoint strata).

Between the two layers the host only concatenates/transposes shards.
"""

import os
from contextlib import ExitStack

import ml_dtypes
import numpy as np

N, E0, DIN, H, DH, DOUT = 50000, 1600000, 128, 8, 16, 7
F1 = H * DH            # 128
F2P = 64               # layer-2 per-node feature block, 8 heads x 8 (7 real)
NCORES = 8
P = 128
NBLK = 392             # 392*128 = 50176 >= N, 392 % 8 == 0
NB = NBLK // NCORES    # 49 blocks per core
NOWN = NB * P          # 6272 nodes per core (incl. pad slots)
NPAD = NBLK * P        # 50176
SPLIT = 32768
TABB_ROWS = NPAD - SPLIT  # 17408
NEG = -1.0e9
EPS = 1e-16

_f32 = np.float32
_bf16 = ml_dtypes.bfloat16


# ---------------------------------------------------------------------------
# host-side graph preprocessing (pure index/layout manipulation)
# ---------------------------------------------------------------------------

def _prep_graph(edge_index):
    src = np.concatenate([edge_index[0], np.arange(N, dtype=np.int64)])
    dst = np.concatenate([edge_index[1], np.arange(N, dtype=np.int64)])
    src = src.astype(np.int64)
    dst = dst.astype(np.int64)

    low = src < SPLIT
    l_cnt = np.bincount(dst[low], minlength=N).astype(np.int64)
    h_cnt = np.bincount(dst[~low], minlength=N).astype(np.int64)

    # group nodes into blocks of 128 with near-equal (low-deg, high-deg)
    order = np.lexsort((h_cnt, l_cnt))
    nodes_sorted = np.concatenate([order, np.full(NPAD - N, -1, np.int64)])
    blocks = nodes_sorted.reshape(NBLK, P)          # [392, 128]

    l_blk = np.where(blocks >= 0, l_cnt[np.maximum(blocks, 0)], 0).max(axis=1)
    h_blk = np.where(blocks >= 0, h_cnt[np.maximum(blocks, 0)], 0).max(axis=1)
    # block-slot j on every core runs global blocks j*8+k; shared strata counts
    GA = l_blk.reshape(NB, NCORES).max(axis=1).astype(int)   # [49]
    GB = h_blk.reshape(NB, NCORES).max(axis=1).astype(int)
    # round up to even so dma_gather's num_idxs register values stay few
    # (bass caches one Pool register per distinct immediate)
    GA = np.where(GA > 0, (GA + 1) & ~1, 0)
    GB = np.where(GB > 0, (GB + 1) & ~1, 0)

    # per-node padded src lists, split by src half
    key = dst * 2 + (~low).astype(np.int64)
    oe = np.argsort(key, kind="stable")
    ss, sk = src[oe], key[oe]
    starts = np.searchsorted(sk, np.arange(2 * N))
    pos = np.arange(len(ss)) - starts[sk]
    Amax = max(int(l_cnt.max()), int(GA.max()))
    Bmax = max(int(h_cnt.max()), int(GB.max()))
    A_pad = np.zeros((N, Amax), np.int32)
    B_pad = np.zeros((N, Bmax), np.int32)
    am = (sk % 2) == 0
    A_pad[sk[am] // 2, pos[am]] = ss[am]
    B_pad[sk[~am] // 2, pos[~am]] = ss[~am] - SPLIT

    sumGA, sumGB = int(GA.sum()), int(GB.sum())
    sumG = sumGA + sumGB

    members = [None] * NCORES
    idxA = [None] * NCORES
    idxB = [None] * NCORES
    mneg = [None] * NCORES
    cnts = [None] * NCORES

    for k in range(NCORES):
        mem = blocks[np.arange(NB) * NCORES + k]       # [49, 128]
        members[k] = mem
        ia = np.zeros((P, 8 * sumGA), np.int16)
        ib = np.zeros((P, 8 * sumGB), np.int16)
        mg = np.full((P, sumG), NEG, _f32)
        cn = np.zeros((1, 2 * NB), np.int32)
        oa = ob = om = 0
        for j in range(NB):
            ga, gb = GA[j], GB[j]
            m = mem[j]
            msafe = np.maximum(m, 0)
            larr = np.where(m >= 0, l_cnt[msafe], 0)
            harr = np.where(m >= 0, h_cnt[msafe], 0)
            # this core's own needed strata; trailing strata get idx -1 so the
            # gather ucode trims them (descgen cost becomes per-core, not the
            # SPMD max across cores)
            own_a = int(larr.max()) if ga else 0
            own_b = int(harr.max()) if gb else 0
            cn[0, 2 * j] = P * own_a
            cn[0, 2 * j + 1] = P * own_b
            if ga:
                plane = A_pad[msafe, :ga]              # [128, ga] (d, q)
                flat = plane.T.reshape(-1)             # slot-major (q, d)
                blk = np.tile(flat.reshape(-1, 16).T, (8, 1)).astype(np.int16)
                blk[:, 8 * own_a:] = -1
                ia[:, 8 * oa:8 * (oa + ga)] = blk
                mg[:, om:om + ga] = np.where(
                    np.arange(ga)[None, :] < larr[:, None], 0.0, NEG)
            if gb:
                plane = B_pad[msafe, :gb]
                flat = plane.T.reshape(-1)
                blk = np.tile(flat.reshape(-1, 16).T, (8, 1)).astype(np.int16)
                blk[:, 8 * own_b:] = -1
                ib[:, 8 * ob:8 * (ob + gb)] = blk
                mg[:, om + ga:om + ga + gb] = np.where(
                    np.arange(gb)[None, :] < harr[:, None], 0.0, NEG)
            oa += ga
            ob += gb
            om += ga + gb
        idxA[k], idxB[k], mneg[k], cnts[k] = ia, ib, mg, cn

    return dict(members=members, GA=GA, GB=GB, idxA=idxA, idxB=idxB,
                mneg=mneg, cnts=cnts, sumGA=sumGA, sumGB=sumGB, sumG=sumG)


# ---------------------------------------------------------------------------
# NEFF builders
# ---------------------------------------------------------------------------

def _mk_bass():
    import concourse.bacc as bacc
    return bacc.Bacc("TRN2", target_bir_lowering=False)


def _build_transform(fo, xl_cols, xl_w, xr_w, fast_mm):
    """xT [128, NOWN] @ Wcat [128, fo] -> xl rows (bf16) + xr rows (bf16).

    xl tensor is [NOWN, xl_cols]; only cols [0:xl_w] are written (the
    gather consumers never read the rest).  xr tensor is [NOWN, xr_w].
    All DMA is batched: one big input load, one big store per output."""
    import concourse.mybir as mybir
    import concourse.tile as tile

    nc = _mk_bass()
    BF16, F32 = mybir.dt.bfloat16, mybir.dt.float32
    xT = nc.dram_tensor("xT", [P, NOWN], F32, kind="ExternalInput")
    W = nc.dram_tensor("Wcat", [P, fo], F32, kind="ExternalInput")
    B = nc.dram_tensor("Bcat", [P, fo], F32, kind="ExternalInput")
    xl = nc.dram_tensor("xl", [NOWN, xl_cols], BF16, kind="ExternalOutput")
    xr = nc.dram_tensor("xr", [NOWN, xr_w], BF16, kind="ExternalOutput")

    with tile.TileContext(nc) as tc, ExitStack() as ctx:
        const = ctx.enter_context(tc.tile_pool(name="const", bufs=1))
        psum = ctx.enter_context(tc.tile_pool(name="psum", bufs=4, space="PSUM"))

        w_f = const.tile([P, fo], F32)
        nc.sync.dma_start(w_f[:], W[:, :])
        w_s = const.tile([P, fo], BF16)
        nc.vector.tensor_copy(w_s[:], w_f[:])
        b_s = const.tile([P, fo], F32)
        nc.sync.dma_start(b_s[:], B[:, :])

        CH = 4                                  # load xT in CH chunks
        NCHUNK = NOWN // CH
        x_f = const.tile([P, NOWN], F32)
        x_b = const.tile([P, NOWN], BF16)
        for c in range(CH):
            sl = slice(c * NCHUNK, (c + 1) * NCHUNK)
            nc.sync.dma_start(x_f[:, sl], xT[:, sl])
            nc.vector.tensor_copy(x_b[:, sl], x_f[:, sl])

        o_l = const.tile([P, NB, xl_w], BF16)
        o_r = const.tile([P, NB, xr_w], BF16)
        for j in range(NB):
            ps = psum.tile([P, fo], F32, tag="ps")
            nc.tensor.matmul(ps[:], x_b[:, j * P:(j + 1) * P], w_s[:],
                             start=True, stop=True)
            nc.vector.tensor_tensor(o_l[:, j, :], ps[:, 0:xl_w],
                                    b_s[:, 0:xl_w], mybir.AluOpType.add)
            nc.vector.tensor_tensor(o_r[:, j, :], ps[:, xl_w:fo],
                                    b_s[:, xl_w:fo], mybir.AluOpType.add)
        # [P, NB, w] sbuf -> [NB, P, w] rows in DRAM, one DMA each
        xl_v = xl[:, 0:xl_w].rearrange("(b p) c -> p b c", p=P)
        nc.sync.dma_start(xl_v, o_l[:])
        xr_v = xr[:, :].rearrange("(b p) c -> p b c", p=P)
        nc.sync.dma_start(xr_v, o_r[:])
    nc.compile()
    return nc


def _build_edge(layer, GA, GB, sumGA, sumGB, sumG):
    """Edge phase for one layer (see module docstring)."""
    import concourse.bass as bass
    import concourse.mybir as mybir
    import concourse.tile as tile
    from concourse import library_config

    FU = F1 if layer == 1 else F2P      # used feature cols (128 / 64)
    C = DH if layer == 1 else 8         # per-head cols in slab (16 / 8)
    FM = FU + H                         # matmul rhs cols (agg | denom)
    FOUT = F1 if layer == 1 else H * DOUT

    nc = _mk_bass()
    dt = mybir.dt
    op = mybir.AluOpType
    AF = mybir.ActivationFunctionType

    tabA = nc.dram_tensor("tabA", [SPLIT, P], dt.bfloat16, kind="ExternalInput")
    tabB = nc.dram_tensor("tabB", [TABB_ROWS, P], dt.bfloat16, kind="ExternalInput")
    xr_d = nc.dram_tensor("xr", [NOWN, FU], dt.bfloat16, kind="ExternalInput")
    idxA = nc.dram_tensor("idxA", [P, 8 * sumGA], dt.int16, kind="ExternalInput")
    idxB = nc.dram_tensor("idxB", [P, 8 * sumGB], dt.int16, kind="ExternalInput")
    mneg = nc.dram_tensor("mneg", [P, sumG], dt.float32, kind="ExternalInput")
    cnts = nc.dram_tensor("cnts", [1, 2 * NB], dt.int32, kind="ExternalInput")
    attT = nc.dram_tensor("attT", [P, FU], dt.bfloat16, kind="ExternalInput")
    biasT = nc.dram_tensor("biasT", [P, FU], dt.float32, kind="ExternalInput")
    idT = nc.dram_tensor("idT", [P, P], dt.bfloat16, kind="ExternalInput")
    out_d = nc.dram_tensor("out", [NOWN, FOUT], dt.float32, kind="ExternalOutput")

    GMAX = int(max(GA[j] + GB[j] for j in range(NB)))

    with tile.TileContext(nc) as tc, ExitStack() as ctx:
        const = ctx.enter_context(tc.tile_pool(name="const", bufs=1))
        io = ctx.enter_context(tc.tile_pool(name="io", bufs=4))
        gpool = ctx.enter_context(tc.tile_pool(name="gpool", bufs=3))
        slabs = ctx.enter_context(tc.tile_pool(name="slabs", bufs=2))
        psum = ctx.enter_context(tc.tile_pool(name="psum", bufs=4, space="PSUM"))
        small = ctx.enter_context(tc.tile_pool(name="small", bufs=3))

        nc.gpsimd.load_library(library_config.mlp)

        regcache = {}

        def nreg(v):
            if v not in regcache:
                regcache[v] = nc.gpsimd.to_reg(v)
            return regcache[v]

        att_s = const.tile([P, FU], dt.bfloat16)
        nc.sync.dma_start(att_s[:], attT[:, :])
        bias_s = const.tile([P, FU], dt.float32)
        nc.sync.dma_start(bias_s[:], biasT[:, :])
        id_s = const.tile([P, P], dt.bfloat16)
        nc.sync.dma_start(id_s[:], idT[:, :])
        cnt_s = const.tile([1, 2 * NB], dt.int32)
        nc.sync.dma_start(cnt_s[:], cnts[:, :])
        creg_a = nc.gpsimd.alloc_register("cnt_a")
        creg_b = nc.gpsimd.alloc_register("cnt_b")

        # zero the slab rings once: trailing-trimmed gathers leave untouched
        # strata, which must hold finite smallish values (see mneg masking)
        for _ in range(3):
            sz = gpool.tile([P, GMAX, P], dt.bfloat16, tag="slab")
            nc.vector.memset(sz[:], 0)

        if layer == 2:
            persist = ctx.enter_context(tc.tile_pool(name="persist", bufs=1))
            mx_all = persist.tile([P, NB], dt.float32)
            s_all = persist.tile([P, NB], dt.float32)
            y_tiles = []

        oa = obi = om = orow = 0
        for j in range(NB):
            ga, gb = int(GA[j]), int(GB[j])
            g = ga + gb
            assert g > 0

            xr_b = io.tile([P, FU], dt.bfloat16, tag="xr")
            nc.sync.dma_start(xr_b[:], xr_d[j * P:(j + 1) * P, :])
            mg = io.tile([P, g], dt.float32, tag="mg")
            nc.sync.dma_start(mg[:], mneg[:, om:om + g])

            slab = gpool.tile([P, g, P], dt.bfloat16, tag="slab")
            if ga:
                ia = io.tile([P, 8 * ga], dt.int16, tag="ia")
                nc.sync.dma_start(ia[:], idxA[:, 8 * oa:8 * (oa + ga)])
                # per-core effective count: idx data carries trailing -1
                # strata; the ucode trims them, and the ring accounting
                # must see the same (trimmed) count via the register
                nc.gpsimd.reg_load(creg_a, cnt_s[0:1, 2 * j:2 * j + 1])
                nc.gpsimd.dma_gather(slab[:, 0:ga, :], tabA[:, :], ia[:],
                                     P * ga, creg_a, P,
                                     single_packet=False)
            if gb:
                ib = io.tile([P, 8 * gb], dt.int16, tag="ib")
                nc.sync.dma_start(ib[:], idxB[:, 8 * obi:8 * (obi + gb)])
                nc.gpsimd.reg_load(creg_b, cnt_s[0:1, 2 * j + 1:2 * j + 2])
                nc.gpsimd.dma_gather(slab[:, ga:g, :], tabB[:, :], ib[:],
                                     P * gb, creg_b, P,
                                     single_packet=False)

            sl_u = slab[:, :, 0:FU]
            tt = slabs.tile([P, g, FU], dt.bfloat16, tag="tt")
            nc.vector.tensor_tensor(
                tt[:], sl_u, xr_b[:].unsqueeze(1).to_broadcast([P, g, FU]),
                op.add)
            uu = slabs.tile([P, g, FU], dt.bfloat16, tag="uu")
            nc.vector.scalar_tensor_tensor(uu[:], tt[:], 0.2, tt[:],
                                           op.mult, op.max)
            vv = slabs.tile([P, g, FU], dt.bfloat16, tag="tt")
            nc.vector.tensor_tensor(
                vv[:], uu[:], att_s[:].unsqueeze(1).to_broadcast([P, g, FU]),
                op.mult)

            sc = small.tile([P, g, H], dt.float32, tag="sc")
            nc.vector.tensor_reduce(
                sc[:], vv[:].rearrange("p g (h c) -> p g h c", c=C),
                mybir.AxisListType.X, op.add)
            sc2 = small.tile([P, g, H], dt.float32, tag="sc2")
            nc.vector.tensor_tensor(
                sc2[:], sc[:], mg[:].unsqueeze(2).to_broadcast([P, g, H]),
                op.add)
            mx = small.tile([P, H], dt.float32, tag="mx")
            nc.vector.tensor_reduce(
                mx[:], sc2[:].rearrange("p g h -> p h g"),
                mybir.AxisListType.X, op.max)
            sc3 = small.tile([P, g, H], dt.float32, tag="sc3")
            nc.vector.tensor_tensor(
                sc3[:], sc2[:], mx[:].unsqueeze(1).to_broadcast([P, g, H]),
                op.subtract)

            Ms = slabs.tile([P, g, FM], dt.bfloat16, tag="Ms")
            exv = Ms[:, :, FU:FM]
            nc.scalar.activation(exv, sc3[:], AF.Exp)
            nc.vector.tensor_tensor(
                Ms[:, :, 0:FU].rearrange("p g (h c) -> p g h c", c=C),
                sl_u.rearrange("p g (h c) -> p g h c", c=C),
                exv.unsqueeze(3).to_broadcast([P, g, H, C]),
                op.mult)

            ps = psum.tile([P, FM], dt.float32, tag="ps")
            for q in range(g):
                nc.tensor.matmul(ps[:], id_s[:], Ms[:, q, :],
                                 start=(q == 0), stop=(q == g - 1))

            dn = small.tile([P, H], dt.float32, tag="dn")
            nc.vector.tensor_scalar_add(dn[:], ps[:, FU:FM], EPS)
            rd = small.tile([P, H], dt.float32, tag="rd")
            nc.vector.reciprocal(rd[:], dn[:])
            ov = small.tile([P, FU], dt.float32, tag="ov")
            nc.vector.tensor_tensor(
                ov[:].rearrange("p (h c) -> p h c", c=C),
                ps[:, 0:FU].rearrange("p (h c) -> p h c", c=C),
                rd[:].unsqueeze(2).to_broadcast([P, H, C]),
                op.mult)
            ob = small.tile([P, FU], dt.float32, tag="ob")
            nc.vector.tensor_tensor(ob[:], ov[:], bias_s[:], op.add)

            if layer == 1:
                mm_t = small.tile([P, FU], dt.float32, tag="mmt")
                nc.vector.tensor_scalar_min(mm_t[:], ob[:], 0.0)
                em = small.tile([P, FU], dt.float32, tag="em")
                nc.scalar.activation(em[:], mm_t[:], AF.Exp)
                hf = small.tile([P, FU], dt.float32, tag="hf")
                nc.vector.scalar_tensor_tensor(
                    hf[:], ob[:], 0.0, em[:], op.max, op.add)
                hg = small.tile([P, FU], dt.float32, tag="hg")
                nc.vector.tensor_scalar_add(hg[:], hf[:], -1.0)
                # store from ACT's HWDGE: keeps the SP queue free for loads
                # (an SP store would head-of-line-block the next blocks' loads)
                nc.scalar.dma_start(out_d[orow:orow + P, :], hg[:])
            else:
                yb = persist.tile([P, FU], dt.float32, tag=f"y{j}",
                                  name=f"y{j}")
                nc.vector.tensor_copy(yb[:], ob[:])
                yr = yb[:].rearrange("p (h c) -> p h c", c=8)[:, :, 0:DOUT]
                mx2 = mx_all[:, j:j + 1]
                nc.vector.tensor_reduce(mx2, yr, mybir.AxisListType.XY,
                                        op.max)
                mxn = small.tile([P, 1], dt.float32, tag="mxn")
                nc.vector.tensor_scalar_mul(mxn[:], mx2, -1.0)
                et = small.tile([P, FOUT], dt.float32, tag="et")
                nc.scalar.activation(
                    et[:].rearrange("p (h c) -> p h c", c=DOUT), yr,
                    AF.Exp, bias=mxn[:])
                nc.vector.tensor_reduce(s_all[:, j:j + 1], et[:],
                                        mybir.AxisListType.X, op.add)
                y_tiles.append(yb)

            oa += ga
            obi += gb
            om += g
            orow += P

        if layer == 2:
            # ln(S) via exponent/mantissa split (no Ln in any HW act table):
            # ln(S) = (e - 127)*ln2 + poly(m), m in [1, 2)
            C5, C4, C3, C2, C1, C0 = (0.030102625011658456,
                                      -0.2806325404494927,
                                      1.1048082361987304,
                                      -2.4208125632180866,
                                      3.4982279012091095,
                                      -1.9316715417207186)
            bits = s_all[:].bitcast(dt.int32)
            ei = persist.tile([P, NB], dt.int32)
            nc.vector.tensor_scalar(ei[:], bits, 23, None,
                                    op.arith_shift_right)
            ef = persist.tile([P, NB], dt.float32)
            nc.vector.tensor_copy(ef[:], ei[:])
            mi = persist.tile([P, NB], dt.int32)
            nc.vector.tensor_scalar(mi[:], bits, 0x007FFFFF, 0x3F800000,
                                    op.bitwise_and, op.bitwise_or)
            mf = mi[:].bitcast(dt.float32)
            pp = persist.tile([P, NB], dt.float32)
            nc.vector.tensor_scalar(pp[:], mf, C5, C4, op.mult, op.add)
            qq = persist.tile([P, NB], dt.float32)
            for ck in (C3, C2, C1, C0):
                nc.vector.tensor_tensor(qq[:], pp[:], mf, op.mult)
                nc.vector.tensor_scalar_add(pp[:], qq[:], ck)
            # ct = mx + (e-127)*ln2 + poly(m)
            lnm = pp
            ct_all = persist.tile([P, NB], dt.float32)
            nc.vector.scalar_tensor_tensor(
                ct_all[:], ef[:], 0.6931471805599453, lnm[:],
                op.mult, op.add)
            ct2 = persist.tile([P, NB], dt.float32)
            nc.vector.scalar_tensor_tensor(
                ct2[:], ct_all[:], -127.0 * 0.6931471805599453, mx_all[:],
                op.add, op.add)
            orow = 0
            for j in range(NB):
                yr = y_tiles[j][:].rearrange("p (h c) -> p h c",
                                             c=8)[:, :, 0:DOUT]
                of = small.tile([P, FOUT], dt.float32, tag="of")
                nc.vector.tensor_scalar_sub(
                    of[:].rearrange("p (h c) -> p h c", c=DOUT), yr,
                    ct2[:, j:j + 1])
                nc.scalar.dma_start(out_d[orow:orow + P, :], of[:])
                orow += P
    nc.compile()
    return nc


# ---------------------------------------------------------------------------
# runner
# ---------------------------------------------------------------------------

_state = {}


def _run(nc, in_maps, trace=False):
    from concourse.bass_utils import run_bass_kernel_spmd
    return run_bass_kernel_spmd(nc, in_maps, core_ids=list(range(NCORES)),
                                trace=trace)


def _bcast_rows(v, rows=P):
    """[n] -> [rows, n] replicated, contiguous."""
    return np.ascontiguousarray(np.broadcast_to(np.asarray(v)[None, :],
                                                (rows, len(v))))


def kernel(x, edge_index, Wl1, bl1, Wr1, br1, att1, bias1,
           Wl2, bl2, Wr2, br2, att2, bias2, _trace=False, _times=None,
           _paths=None):
    x = np.asarray(x, _f32)
    edge_index = np.asarray(edge_index)

    g = _prep_graph(edge_index)
    members, GA, GB = g["members"], g["GA"], g["GB"]

    ckey = (tuple(GA), tuple(GB))
    if _state.get("ckey") != ckey:
        _state["ckey"] = ckey
        _state["nc_t1"] = _build_transform(2 * F1, F1, F1, F1, fast_mm=False)
        _state["nc_t2"] = _build_transform(2 * F2P, P, F2P, F2P, fast_mm=False)
        _state["nc_e1"] = _build_edge(1, GA, GB, g["sumGA"], g["sumGB"], g["sumG"])
        _state["nc_e2"] = _build_edge(2, GA, GB, g["sumGA"], g["sumGB"], g["sumG"])

    id128 = np.eye(P, dtype=_bf16)

    def gather_nodes(arr, mem):
        flat = mem.reshape(-1)
        out = arr[np.maximum(flat, 0)]
        out[flat < 0] = 0
        return out

    def trace_run(key, nc, in_maps):
        r = _run(nc, in_maps, trace=_trace)
        if _times is not None:
            _times[key] = r.exec_time_ns
        if _paths is not None and r.instructions_and_trace is not None:
            _paths[key] = r.instructions_and_trace[1]
        return r.results

    # ---- T1 ----
    W1 = np.concatenate([Wl1, Wr1], axis=1).astype(_f32)       # [128, 256]
    B1 = np.concatenate([bl1, br1]).astype(_f32)               # [256]
    B1t = _bcast_rows(B1)
    t1_maps = []
    for k in range(NCORES):
        xg = gather_nodes(x, members[k])                       # [6272, 128]
        t1_maps.append({"xT": np.ascontiguousarray(xg.T),
                        "Wcat": W1, "Bcat": B1t})
    r1 = trace_run("t1", _state["nc_t1"], t1_maps)

    # assemble layer-1 gather table
    tab1 = np.zeros((NPAD, P), _bf16)
    for k in range(NCORES):
        flat = members[k].reshape(-1)
        ok = flat >= 0
        tab1[flat[ok]] = r1[k]["xl"][ok]
    tab1A = np.ascontiguousarray(tab1[:SPLIT])
    tab1B = np.ascontiguousarray(tab1[SPLIT:])

    # ---- E1 ----
    att1_t = _bcast_rows(att1.reshape(-1)).astype(_bf16)       # [128, 128]
    bias1_t = _bcast_rows(bias1).astype(_f32)
    e1_maps = []
    for k in range(NCORES):
        e1_maps.append({"tabA": tab1A, "tabB": tab1B,
                        "xr": r1[k]["xr"],
                        "idxA": g["idxA"][k], "idxB": g["idxB"][k],
                        "mneg": g["mneg"][k], "cnts": g["cnts"][k],
                        "attT": att1_t, "biasT": bias1_t, "idT": id128})
    re1 = trace_run("e1", _state["nc_e1"], e1_maps)

    # ---- T2 ----
    Wl2p = np.zeros((P, F2P), _f32)
    Wl2p.reshape(P, H, 8)[:, :, :DOUT] = np.asarray(Wl2, _f32).reshape(P, H, DOUT)
    Wr2p = np.zeros((P, F2P), _f32)
    Wr2p.reshape(P, H, 8)[:, :, :DOUT] = np.asarray(Wr2, _f32).reshape(P, H, DOUT)
    W2 = np.ascontiguousarray(np.concatenate([Wl2p, Wr2p], axis=1))  # [128,128]
    bl2p = np.zeros(F2P, _f32)
    bl2p.reshape(H, 8)[:, :DOUT] = np.asarray(bl2, _f32).reshape(H, DOUT)
    br2p = np.zeros(F2P, _f32)
    br2p.reshape(H, 8)[:, :DOUT] = np.asarray(br2, _f32).reshape(H, DOUT)
    B2t = _bcast_rows(np.concatenate([bl2p, br2p]))
    t2_maps = []
    for k in range(NCORES):
        t2_maps.append({"xT": np.ascontiguousarray(re1[k]["out"].T),
                        "Wcat": W2, "Bcat": B2t})
    r2 = trace_run("t2", _state["nc_t2"], t2_maps)

    tab2 = np.zeros((NPAD, P), _bf16)
    for k in range(NCORES):
        flat = members[k].reshape(-1)
        ok = flat >= 0
        tab2[flat[ok]] = r2[k]["xl"][ok]
    tab2A = np.ascontiguousarray(tab2[:SPLIT])
    tab2B = np.ascontiguousarray(tab2[SPLIT:])

    # ---- E2 ----
    att2p = np.zeros((H, 8), _f32)
    att2p[:, :DOUT] = np.asarray(att2, _f32)
    att2_t = _bcast_rows(att2p.reshape(-1)).astype(_bf16)      # [128, 64]
    bias2p = np.zeros(F2P, _f32)
    bias2p.reshape(H, 8)[:, :DOUT] = np.asarray(bias2, _f32).reshape(H, DOUT)
    bias2_t = _bcast_rows(bias2p)
    e2_maps = []
    for k in range(NCORES):
        e2_maps.append({"tabA": tab2A, "tabB": tab2B,
                        "xr": r2[k]["xr"],
                        "idxA": g["idxA"][k], "idxB": g["idxB"][k],
                        "mneg": g["mneg"][k], "cnts": g["cnts"][k],
                        "attT": att2_t, "biasT": bias2_t, "idT": id128})
    re2 = trace_run("e2", _state["nc_e2"], e2_maps)

    out = np.zeros((N, H * DOUT), _f32)
    for k in range(NCORES):
        flat = members[k].reshape(-1)
        ok = flat >= 0
        out[flat[ok]] = re2[k]["out"][ok]
    return out

